# revision 16
# baseline (speedup 1.0000x reference)
"""Trainium2 Bass kernel for nn_Druggability_DistillModel (gnn_message_passing).

Strategy (8 NeuronCores, data-parallel over B x 4-way sequence shards):
  - core c handles batch b=c//4, tokens [s*512, (s+1)*512) with s=c%4.
  - The edge-bias MLP depends only on rel_pos (65 possible values) -> host
    collapses it to a 65-entry table, then builds a dense matrix
    E^T[j, t] = sum_dup exp(edge(t,k)) over neighbors k with idx==j (0 else).
    softmax_k(q.k/16 + edge) * v  ==  (exp(q.hK^T) * E) @ hV / rowsum -
    exact (duplicate neighbor indices merge correctly since exp adds).
    This removes the neighbor gather entirely; the attention is dense PE work.
  - Per-core inputs are token-rotated so the core's shard is always rows
    0:512 -> one SPMD program for all 8 cores.
  - Activations flow in transposed [d, token] layout; matmuls take weights
    as lhsT directly, so no transposes inside chains.
  - Pre-work (LN/hK/hV/kv) and the attention sweep are emitted interleaved
    per 4-tile group, with accumulating matmuls software-pipelined one step
    behind their producers, so PE never stalls on ACT/DVE.
"""
import sys

sys.path.insert(0, "/opt/trn_rl_repo")

import math
import numpy as np
import ml_dtypes

B, L, D, H, DH, K, DE, CLIP = 2, 2048, 256, 8, 32, 36, 64, 32
NCORES, SPB, SH = 8, 4, 512  # cores, shards/batch, tokens/shard
NT = L // 128                # 16 token tiles per batch
ST = SH // 128               # 4 tiles per shard
BF16 = ml_dtypes.bfloat16

_CACHE: dict = {}


def _gelu_np(x):
    try:
        from scipy.special import erf
        e = erf(x / np.sqrt(2.0))
    except Exception:
        import math as _m
        e = np.vectorize(_m.erf)(x / np.sqrt(2.0))
    return x * 0.5 * (1.0 + e)


def _w_tiles(w, cin_chunks):
    """[din, dout] -> [128, cin_chunks, dout] with din = c*128+p."""
    din, dout = w.shape
    assert din == cin_chunks * 128
    return np.ascontiguousarray(
        w.reshape(cin_chunks, 128, dout).transpose(1, 0, 2)
    ).astype(BF16)


def _build(taps=()):
    import concourse.bass as bass
    import concourse.tile as tile
    from concourse import bacc, mybir
    from concourse.masks import make_identity

    f32, bf = mybir.dt.float32, mybir.dt.bfloat16
    AF = mybir.ActivationFunctionType
    ALU = mybir.AluOpType
    AX = mybir.AxisListType

    nc = bacc.Bacc("TRN2", target_bir_lowering=False, debug=False)

    x_d = nc.dram_tensor("x", [L, D], f32, kind="ExternalInput")
    et_d = nc.dram_tensor("et", [L, SH], bf, kind="ExternalInput")
    aff_d = nc.dram_tensor("aff", [128, 2, 4], f32, kind="ExternalInput")
    wq_d = nc.dram_tensor("wq", [128, 2, D], bf, kind="ExternalInput")
    wk_d = nc.dram_tensor("wk", [128, 2, D], bf, kind="ExternalInput")
    wv_d = nc.dram_tensor("wv", [128, 2, D], bf, kind="ExternalInput")
    wg1_d = nc.dram_tensor("wg1", [128, 4, D], bf, kind="ExternalInput")
    wg2_d = nc.dram_tensor("wg2", [128, 2, D], bf, kind="ExternalInput")
    wqkv_d = nc.dram_tensor("wqkv", [128, 2, 3 * D], bf, kind="ExternalInput")
    wgo_d = nc.dram_tensor("wgo", [128, 2, D], bf, kind="ExternalInput")
    wf1_d = nc.dram_tensor("wf1", [128, 2, D], bf, kind="ExternalInput")
    wf2_d = nc.dram_tensor("wf2", [128, 2, 2], bf, kind="ExternalInput")
    wff1_d = nc.dram_tensor("wff1", [128, 2, 4 * D], bf, kind="ExternalInput")
    wff2_d = nc.dram_tensor("wff2", [128, 8, D], bf, kind="ExternalInput")
    pm_d = nc.dram_tensor("pm", [2, 1], bf, kind="ExternalInput")
    out_d = nc.dram_tensor("out", [SH, D], f32, kind="ExternalOutput")
    tap_tiles = {}

    with tile.TileContext(nc) as tc:
        with (
            tc.tile_pool(name="const", bufs=1) as const,
            tc.tile_pool(name="persist", bufs=1) as pers,
            tc.tile_pool(name="stream", bufs=3) as stm,
            tc.tile_pool(name="stmf", bufs=6) as stmf,
            tc.tile_pool(name="stmq", bufs=8) as stmq,
            tc.tile_pool(name="psmm", bufs=3, space="PSUM") as psmm,
            tc.tile_pool(name="psacc", bufs=4, space="PSUM") as psacc,
            tc.tile_pool(name="pssml", bufs=1, space="PSUM") as pssml,
        ):
            ident = const.tile([128, 128], f32)
            make_identity(nc, ident[:])
            ones_cb = const.tile([128, 1], bf)
            nc.vector.memset(ones_cb[:], 1.0)
            ones_rb = const.tile([1, 128], bf)
            nc.vector.memset(ones_rb[:], 1.0)
            pm = const.tile([2, 1], bf)
            nc.sync.dma_start(pm[:], pm_d[:])
            eps5 = const.tile([128, 1], f32)
            nc.vector.memset(eps5[:], 1e-5)
            aff = const.tile([128, 2, 4], f32)
            nc.sync.dma_start(aff[:], aff_d[:])

            def wload(dram, shape):
                t = const.tile(list(shape), bf, tag=dram.name)
                nc.sync.dma_start(t[:], dram[:])
                return t


            # first weights PE will need, then x chunks (LN starts on x0)
            wv = wload(wv_d, (128, 2, D))
            wqkv = wload(wqkv_d, (128, 2, 3 * D))
            x_all = pers.tile([128, NT, D], f32)
            x_r = x_d.rearrange("(n p) d -> p n d", p=128)
            for qg_ in range(4):
                nc.sync.dma_start(x_all[:, qg_ * 4:(qg_ + 1) * 4, :],
                                  x_r[:, qg_ * 4:(qg_ + 1) * 4, :])
            wk = wload(wk_d, (128, 2, D))
            wq = wload(wq_d, (128, 2, D))
            et_r = et_d.rearrange("(n p) t -> p n t", p=128)
            et_all = pers.tile([128, NT, SH], bf)
            for qg_ in range(4):
                nc.sync.dma_start(et_all[:, qg_ * 4:(qg_ + 1) * 4, :],
                                  et_r[:, qg_ * 4:(qg_ + 1) * 4, :])
            wf1 = wload(wf1_d, (128, 2, D))
            wf2 = wload(wf2_d, (128, 2, 2))
            wg1 = wload(wg1_d, (128, 4, D))
            wg2 = wload(wg2_d, (128, 2, D))
            wgo = wload(wgo_d, (128, 2, D))
            wff1 = wload(wff1_d, (128, 2, 4 * D))
            wff2 = wload(wff2_d, (128, 8, D))

            hT = pers.tile([128, 2, L], bf)    # h^T, full batch
            hKT = pers.tile([128, 2, L], bf)   # (h@Wk)^T, full batch
            hV = pers.tile([128, NT, D], bf)   # h@Wv, token-major, full batch
            tap_tiles["hT"], tap_tiles["hKT"], tap_tiles["hV"] = hT, hKT, hV
            qT = pers.tile([128, 2, SH], bf)
            tap_tiles["qT"] = qT
            den_sb = pers.tile([1, 512], f32)
            nc.vector.memset(den_sb[:], 0.0)

            f1T = pers.tile([128, 2, SH], bf)
            wf_ps = pssml.tile([2, 512], f32, tag="accs")
            wf0 = pers.tile([1, 512], bf)
            tap_tiles["wf0"] = wf0
            wf1s = pers.tile([1, 512], bf)
            qg_all = pers.tile([128, ST, D], f32)
            kv_ps = [psacc.tile([128, 257], f32, tag="acc", name=f"kv{g}") for g in range(2)]
            agg_ps = [psacc.tile([128, 512], f32, tag="acc", name=f"agg{g}") for g in range(2)]

            # producers for the software-pipelined accumulators
            kg_tiles = {}
            ut_tiles = {}

            def emit_kv(n):
                kg_l, vg_rhs = kg_tiles.pop(n)
                for g in range(2):
                    nc.tensor.matmul(kv_ps[g][:], kg_l[:, g * 128:(g + 1) * 128],
                                     vg_rhs[:], start=(n == 0), stop=(n == NT - 1))

            def emit_attn_acc(jc):
                ut = ut_tiles.pop(jc)
                dch = psmm.tile([1, 512], f32, tag="mm")
                nc.tensor.matmul(dch[:], ones_cb[:], ut[:], start=True, stop=True)
                nc.vector.tensor_add(den_sb[:], den_sb[:], dch[:])
                for g in range(2):
                    nc.tensor.matmul(agg_ps[g][:], hV[:, jc, g * 128:(g + 1) * 128],
                                     ut[:], start=(jc == 0), stop=(jc == NT - 1))

            # ---------- fused pre-work + attention, per 4-tile group ----------
            for qgrp in range(4):
                tiles = range(qgrp * 4, qgrp * 4 + 4)
                mval = stm.tile([128, 4, 2], f32, tag="mval")
                for i, n in enumerate(tiles):
                    stats = stm.tile([128, 6], f32, tag="stats")
                    nc.vector.bn_stats(out=stats[:], in_=x_all[:, n, :])
                    nc.vector.bn_aggr(out=mval[:, i, :], in_=stats[:])
                sd4 = stm.tile([128, 4], f32, tag="sd4")
                nc.scalar.activation(sd4[:], mval[:, :, 1], AF.Sqrt, bias=eps5[:, 0:1])
                rstd4 = stm.tile([128, 4], f32, tag="rstd4")
                nc.vector.reciprocal(rstd4[:], sd4[:])
                nmr4 = stm.tile([128, 4], f32, tag="nmr4")
                nc.vector.tensor_mul(nmr4[:], mval[:, :, 0], rstd4[:])
                nc.vector.tensor_scalar_mul(nmr4[:], nmr4[:], -1.0)
                for i, n in enumerate(tiles):
                    js = slice(n * 128, (n + 1) * 128)
                    # hn = (x - m) * rstd  (one fused DVE op)
                    hn = stmq.tile([128, D], f32, tag="tmpq")
                    nc.vector.tensor_scalar(hn[:], x_all[:, n, :],
                                            rstd4[:, i:i + 1], nmr4[:, i:i + 1],
                                            op0=ALU.mult, op1=ALU.add)
                    for c in range(2):
                        pt = psmm.tile([128, 128], f32, tag="mm")
                        nc.tensor.transpose(pt[:], hn[:, c * 128:(c + 1) * 128], ident[:])
                        # h = hn * g1 + b1 on the transposed copy-out (DVE)
                        nc.vector.tensor_scalar(hT[:, c, js], pt[:],
                                                aff[:, c, 0:1], aff[:, c, 1:2],
                                                op0=ALU.mult, op1=ALU.add)
                    # hV tile
                    pv = psmm.tile([128, D], f32, tag="mm")
                    for c in range(2):
                        nc.tensor.matmul(pv[:], hT[:, c, js], wv[:, c, :],
                                         start=(c == 0), stop=(c == 1))
                    nc.scalar.copy(hV[:, n, :], pv[:])
                    # kg/vg projection + elu(k)+1
                    pq = psmm.tile([128, 512], f32, tag="mm")
                    for c in range(2):
                        nc.tensor.matmul(pq[:], hT[:, c, js], wqkv[:, c, D:3 * D],
                                         start=(c == 0), stop=(c == 1))
                    t0 = stmq.tile([128, D], f32, tag="tmpq")
                    nc.vector.tensor_scalar_min(t0[:], pq[:, 0:D], 0.0)
                    te = stmq.tile([128, D], f32, tag="tmpq")
                    nc.scalar.activation(te[:], t0[:], AF.Exp)
                    tr = stmq.tile([128, D], f32, tag="tmpq")
                    nc.vector.tensor_sub(tr[:], pq[:, 0:D], t0[:])  # relu = x - min(x,0)
                    kg_l = stm.tile([128, D], bf, tag="kg_l")
                    nc.gpsimd.tensor_add(kg_l[:], te[:], tr[:])
                    vg_rhs = stm.tile([128, D + 1], bf, tag="vg_rhs")
                    nc.scalar.copy(vg_rhs[:, 0:D], pq[:, D:2 * D])
                    nc.gpsimd.memset(vg_rhs[:, D:D + 1], 1.0)
                    kg_tiles[n] = (kg_l, vg_rhs)
                    if n >= 1:
                        emit_kv(n - 1)

                # hKT chunk for this group
                jsg = slice(qgrp * 512, (qgrp + 1) * 512)
                for g in range(2):
                    pk = psmm.tile([128, 512], f32, tag="mm")
                    for c in range(2):
                        nc.tensor.matmul(pk[:], wk[:, c, g * 128:(g + 1) * 128],
                                         hT[:, c, jsg], start=(c == 0), stop=(c == 1))
                    nc.scalar.copy(hKT[:, g, jsg], pk[:])
                # qT (needs hT tiles 0..3 only)
                if qgrp == 0:
                    for g in range(2):
                        pq2 = psmm.tile([128, 512], f32, tag="mm")
                        for c in range(2):
                            nc.tensor.matmul(pq2[:], wq[:, c, g * 128:(g + 1) * 128],
                                             hT[:, c, 0:SH], start=(c == 0), stop=(c == 1))
                        nc.vector.tensor_copy(qT[:, g, :], pq2[:])
                    # early independent chains: f1/wf gate and linear-attn qg
                    for g in range(2):
                        pf = psmm.tile([128, 512], f32, tag="mm")
                        for c in range(2):
                            nc.tensor.matmul(pf[:], wf1[:, c, g * 128:(g + 1) * 128],
                                             hT[:, c, 0:SH], start=(c == 0), stop=(c == 1))
                        nc.scalar.activation(f1T[:, g, :], pf[:], AF.Gelu)
                    for c in range(2):
                        nc.tensor.matmul(wf_ps[:], wf2[:, c, :], f1T[:, c, :],
                                         start=(c == 0), stop=(c == 1))
                    wf_sb = stm.tile([2, 512], bf, tag="wf_sb")
                    nc.scalar.copy(wf_sb[:], wf_ps[:])
                    d01_ps = pssml.tile([1, 512], f32, tag="accs")
                    nc.tensor.matmul(d01_ps[:], pm[:], wf_sb[:], start=True, stop=True)
                    nc.scalar.activation(wf0[:], d01_ps[:], AF.Sigmoid)
                    nc.scalar.activation(wf1s[:], d01_ps[:], AF.Sigmoid, scale=-1.0)
                    for it in range(ST):
                        ts_ = slice(it * 128, (it + 1) * 128)
                        pq3 = psmm.tile([128, D], f32, tag="mm")
                        for c in range(2):
                            nc.tensor.matmul(pq3[:], hT[:, c, ts_], wqkv[:, c, 0:D],
                                             start=(c == 0), stop=(c == 1))
                        t0b = stmq.tile([128, D], f32, tag="tmpq")
                        nc.vector.tensor_scalar_min(t0b[:], pq3[:], 0.0)
                        teb = stmq.tile([128, D], f32, tag="tmpq")
                        nc.scalar.activation(teb[:], t0b[:], AF.Exp)
                        trb = stmq.tile([128, D], f32, tag="tmpq")
                        nc.vector.tensor_sub(trb[:], pq3[:], t0b[:])
                        nc.vector.tensor_add(qg_all[:, it, :], teb[:], trb[:])

                # attention chunks for this group (acc pipelined one behind)
                for jc in tiles:
                    js = slice(jc * 128, (jc + 1) * 128)
                    pl = psmm.tile([128, 512], f32, tag="mm")
                    for c in range(2):
                        nc.tensor.matmul(pl[:], hKT[:, c, js], qT[:, c, :],
                                         start=(c == 0), stop=(c == 1))
                    ex = stm.tile([128, 512], bf, tag="ex")
                    nc.scalar.activation(ex[:], pl[:], AF.Exp)
                    ut = stm.tile([128, 512], bf, tag="ut")
                    nc.gpsimd.tensor_mul(ut[:], ex[:], et_all[:, jc, :])
                    ut_tiles[jc] = ut
                    if jc >= 1:
                        emit_attn_acc(jc - 1)
            emit_kv(NT - 1)
            emit_attn_acc(NT - 1)

            # ---------- kv block-diagonal matrix + ksum row ----------
            kvb = pers.tile([128, 2, D], bf)
            tap_tiles["kvb"] = kvb
            nc.vector.memset(kvb[:], 0.0)
            for h in range(H):
                g, po = h // 4, (h * DH) % 128
                nc.scalar.copy(kvb[po:po + DH, g, h * DH:(h + 1) * DH],
                               kv_ps[g][po:po + DH, h * DH:(h + 1) * DH])
            ksum_col = pers.tile([128, 2], f32)
            for g in range(2):
                nc.vector.tensor_copy(ksum_col[:, g:g + 1], kv_ps[g][:, D:D + 1])
            ksum_row = pers.tile([1, D], bf)
            for g in range(2):
                pt = psmm.tile([128, 128], f32, tag="mm")
                nc.tensor.transpose(pt[0:1, 0:128], ksum_col[:, g:g + 1], ident[:])
                nc.vector.tensor_copy(ksum_row[0:1, g * 128:(g + 1) * 128], pt[0:1, 0:128])
            kb_ps = psmm.tile([128, D], f32, tag="mm")
            nc.tensor.matmul(kb_ps[:], ones_rb[:], ksum_row[:], start=True, stop=True)
            ksumb = pers.tile([128, D], bf)
            tap_tiles["ksumb"] = ksumb
            nc.vector.tensor_copy(ksumb[:], kb_ps[:])

            # ---------- attention normalize (agg matmuls already include Wlo) --
            den_r = pers.tile([1, 512], bf)
            tap_tiles["den_r"] = den_r
            with nc.allow_low_precision("bf16 recip feeds bf16 broadcast"):
                nc.vector.reciprocal(den_r[:], den_sb[:])   # f32 in -> bf16 out

            # tail processed in two 256-token halves, emitted stage-major so
            # the scheduler pipelines half 1's stage k with half 0's stage k+1
            HH = SH // 2  # 256
            aggloT = pers.tile([128, 2, SH], bf)
            tap_tiles["aggloT"] = aggloT
            g1T = pers.tile([128, 2, SH], bf)
            gateT = pers.tile([128, 2, SH], bf)
            tap_tiles["gateT"] = gateT
            h_localT = pers.tile([128, 2, SH], f32)
            tap_tiles["h_localT"] = h_localT
            qgzT = pers.tile([128, 2, SH], bf)
            tap_tiles["qgzT"] = qgzT
            yT = pers.tile([128, 2, SH], bf)
            tap_tiles["yT"] = yT
            h_globalT = pers.tile([128, 2, SH], f32)
            tap_tiles["h_globalT"] = h_globalT
            xoT = pers.tile([128, 2, SH], f32)
            tap_tiles["xoT"] = xoT
            xo_bf = pers.tile([128, 2, SH], bf)
            xnT = pers.tile([128, 2, SH], bf)
            tap_tiles["xnT"] = xnT
            ff1T = pers.tile([128, 8, SH], bf)
            outT = pers.tile([128, 2, SH], f32)
            tap_tiles["outT"] = outT
            HS = [slice(0, HH), slice(HH, SH)]

            # rb broadcast per half
            rb_sb = []
            for hh in range(2):
                rbp = psmm.tile([128, HH], f32, tag="mm", name=f"rbp{hh}")
                nc.tensor.matmul(rbp[:], ones_rb[:], den_r[0:1, HS[hh]],
                                 start=True, stop=True)
                rbh = stmf.tile([128, HH], f32, tag="tmpf", name=f"rb{hh}")
                nc.scalar.copy(rbh[:], rbp[:])
                rb_sb.append(rbh)
            # agglo = agg_un * recip(den)
            for hh in range(2):
                for g in range(2):
                    nc.vector.tensor_mul(aggloT[:, g, HS[hh]],
                                         agg_ps[g][:, HS[hh]], rb_sb[hh][:])
            # gate chain
            for hh in range(2):
                for g in range(2):
                    pool_ = psacc if hh else psmm
                    pg = pool_.tile([128, HH], f32, tag="acc" if hh else "mm", name=f"pg{hh}{g}")
                    for c in range(2):
                        nc.tensor.matmul(pg[:], wg1[:, c, g * 128:(g + 1) * 128],
                                         hT[:, c, HS[hh]], start=(c == 0), stop=False)
                    for c in range(2):
                        nc.tensor.matmul(pg[:], wg1[:, 2 + c, g * 128:(g + 1) * 128],
                                         aggloT[:, c, HS[hh]], start=False, stop=(c == 1))
                    nc.scalar.activation(g1T[:, g, HS[hh]], pg[:], AF.Gelu)
            for hh in range(2):
                for g in range(2):
                    pool_ = psacc if hh else psmm
                    pg2 = pool_.tile([128, HH], f32, tag="acc" if hh else "mm", name=f"pg2{hh}{g}")
                    for c in range(2):
                        nc.tensor.matmul(pg2[:], wg2[:, c, g * 128:(g + 1) * 128],
                                         g1T[:, c, HS[hh]], start=(c == 0), stop=(c == 1))
                    nc.scalar.activation(gateT[:, g, HS[hh]], pg2[:], AF.Sigmoid)
            for hh in range(2):
                for c in range(2):
                    tl = stmf.tile([128, HH], f32, tag="tmpf")
                    nc.gpsimd.tensor_mul(tl[:], gateT[:, c, HS[hh]], aggloT[:, c, HS[hh]])
                    nc.gpsimd.tensor_add(h_localT[:, c, HS[hh]], tl[:], hT[:, c, HS[hh]])

            # linear attention z + qgz (qg computed early; half hh = tiles 2hh,2hh+1)
            zden_a = stm.tile([128, ST, H], f32, tag="zden_a")
            for it in range(ST):
                prod = stmq.tile([128, D], f32, tag="tmpq")
                nc.vector.tensor_mul(prod[:], qg_all[:, it, :], ksumb[:])
                nc.vector.tensor_reduce(zden_a[:, it, :],
                                        prod[:].rearrange("p (h d) -> p h d", d=DH),
                                        axis=AX.X, op=ALU.add)
            nc.vector.tensor_scalar_add(zden_a[:], zden_a[:], 1e-6)
            zr_a = stm.tile([128, ST, H], f32, tag="zr_a")
            nc.vector.reciprocal(zr_a[:], zden_a[:])
            for it in range(ST):
                ts_ = slice(it * 128, (it + 1) * 128)
                qgz = stmq.tile([128, D], f32, tag="tmpq")
                nc.gpsimd.tensor_tensor(
                    out=qgz[:].rearrange("p (h d) -> p h d", d=DH),
                    in0=qg_all[:, it, :].rearrange("p (h d) -> p h d", d=DH),
                    in1=zr_a[:, it, :].to_broadcast([128, H, DH]), op=ALU.mult)
                for c in range(2):
                    ptq = psmm.tile([128, 128], f32, tag="mm")
                    nc.tensor.transpose(ptq[:], qgz[:, c * 128:(c + 1) * 128], ident[:])
                    nc.scalar.copy(qgzT[:, c, ts_], ptq[:])
            for hh in range(2):
                for g in range(2):
                    pool_ = psacc if hh else psmm
                    py = pool_.tile([128, HH], f32, tag="acc" if hh else "mm", name=f"py{hh}{g}")
                    for c in range(2):
                        nc.tensor.matmul(py[:], kvb[:, c, g * 128:(g + 1) * 128],
                                         qgzT[:, c, HS[hh]], start=(c == 0), stop=(c == 1))
                    nc.vector.tensor_copy(yT[:, g, HS[hh]], py[:])
            for hh in range(2):
                for g in range(2):
                    pool_ = psacc if hh else psmm
                    pgo = pool_.tile([128, HH], f32, tag="acc" if hh else "mm", name=f"pgo{hh}{g}")
                    for c in range(2):
                        nc.tensor.matmul(pgo[:], wgo[:, c, g * 128:(g + 1) * 128],
                                         yT[:, c, HS[hh]], start=(c == 0), stop=(c == 1))
                    nc.vector.tensor_add(h_globalT[:, g, HS[hh]], hT[:, g, HS[hh]], pgo[:])

            # xo = wf0*h_local + wf1*h_global  (wf0/wf1s computed early)
            for hh in range(2):
                b0p = psmm.tile([128, HH], f32, tag="mm", name=f"b0p{hh}")
                nc.tensor.matmul(b0p[:], ones_rb[:], wf0[0:1, HS[hh]], start=True, stop=True)
                b1p = psmm.tile([128, HH], f32, tag="mm", name=f"b1p{hh}")
                nc.tensor.matmul(b1p[:], ones_rb[:], wf1s[0:1, HS[hh]], start=True, stop=True)
                for c in range(2):
                    ta = stmf.tile([128, HH], f32, tag="tmpf")
                    nc.vector.tensor_mul(ta[:], h_localT[:, c, HS[hh]], b0p[:])
                    tb = stmf.tile([128, HH], f32, tag="tmpf")
                    nc.vector.tensor_mul(tb[:], h_globalT[:, c, HS[hh]], b1p[:])
                    nc.vector.tensor_add(xoT[:, c, HS[hh]], ta[:], tb[:])
                    nc.gpsimd.tensor_copy(xo_bf[:, c, HS[hh]], xoT[:, c, HS[hh]])

            # LN2 stats per half (bf16 sums via ones matmul)
            for hh in range(2):
                sum_ps = pssml.tile([1, HH], f32, tag="accs")
                for c in range(2):
                    nc.tensor.matmul(sum_ps[:], ones_cb[:], xo_bf[:, c, HS[hh]],
                                     start=(c == 0), stop=(c == 1))
                ssq_ps = psmm.tile([1, HH], f32, tag="mm", name=f"ssq{hh}")
                for c in range(2):
                    xsq = stmf.tile([128, HH], bf, tag="xsq")
                    nc.scalar.activation(xsq[:], xoT[:, c, HS[hh]], AF.Square)
                    nc.tensor.matmul(ssq_ps[:], ones_cb[:], xsq[:],
                                     start=(c == 0), stop=(c == 1))
                mean = stm.tile([1, HH], f32, tag="mean", name=f"mean{hh}")
                nc.scalar.mul(mean[:], sum_ps[:], 1.0 / D)
                mean_b = stm.tile([1, HH], bf, tag="mean_b", name=f"meanb{hh}")
                nc.vector.tensor_copy(mean_b[:], mean[:])
                msq = stm.tile([1, HH], f32, tag="msq")
                nc.scalar.mul(msq[:], ssq_ps[:], 1.0 / D)
                m2 = stm.tile([1, HH], f32, tag="m2")
                nc.vector.tensor_mul(m2[:], mean[:], mean[:])
                var = stm.tile([1, HH], f32, tag="var")
                nc.vector.tensor_sub(var[:], msq[:], m2[:])
                sd2 = stm.tile([1, HH], f32, tag="sd2")
                nc.scalar.activation(sd2[:], var[:], AF.Sqrt, bias=eps5[0:1, 0:1])
                rstd2 = stm.tile([1, HH], bf, tag="rstd2", name=f"rstd2{hh}")
                with nc.allow_low_precision("bf16 recip feeds bf16 broadcast"):
                    nc.vector.reciprocal(rstd2[:], sd2[:])
                mb_ps = psmm.tile([128, HH], f32, tag="mm", name=f"mb{hh}")
                nc.tensor.matmul(mb_ps[:], ones_rb[:], mean_b[:], start=True, stop=True)
                rb2_ps = psmm.tile([128, HH], f32, tag="mm", name=f"rb2{hh}")
                nc.tensor.matmul(rb2_ps[:], ones_rb[:], rstd2[:], start=True, stop=True)
                for c in range(2):
                    t1 = stmf.tile([128, HH], f32, tag="tmpf")
                    nc.vector.tensor_sub(t1[:], xoT[:, c, HS[hh]], mb_ps[:])
                    # g2 is folded into Wff1 on the host; write bf16 directly
                    nc.vector.tensor_mul(xnT[:, c, HS[hh]], t1[:], rb2_ps[:])

            # FFN + residual, halved
            for hh in range(2):
                for g8 in range(8):
                    pool_ = psacc if hh else psmm
                    pff = pool_.tile([128, HH], f32, tag="acc" if hh else "mm", name=f"pff{hh}")
                    for c in range(2):
                        nc.tensor.matmul(pff[:], wff1[:, c, g8 * 128:(g8 + 1) * 128],
                                         xnT[:, c, HS[hh]], start=(c == 0), stop=(c == 1))
                    nc.scalar.activation(ff1T[:, g8, HS[hh]], pff[:], AF.Gelu)
            for hh in range(2):
                for g in range(2):
                    pool_ = psacc if hh else psmm
                    pf2 = pool_.tile([128, HH], f32, tag="acc" if hh else "mm", name=f"pf2{hh}{g}")
                    for c8 in range(8):
                        nc.tensor.matmul(pf2[:], wff2[:, c8, g * 128:(g + 1) * 128],
                                         ff1T[:, c8, HS[hh]], start=(c8 == 0), stop=(c8 == 7))
                    nc.vector.tensor_add(outT[:, g, HS[hh]], xoT[:, g, HS[hh]], pf2[:])

            # transpose out + store
            for it in range(ST):
                ts_ = slice(it * 128, (it + 1) * 128)
                ot = stmq.tile([128, D], f32, tag="tmpq")
                for c in range(2):
                    pto = psmm.tile([128, 128], f32, tag="mm")
                    nc.tensor.transpose(pto[:], outT[:, c, ts_], ident[:])
                    nc.scalar.copy(ot[:, c * 128:(c + 1) * 128], pto[:])
                nc.sync.dma_start(out_d[ts_, :], ot[:])

            for name in taps:
                t = tap_tiles[name]
                td = nc.dram_tensor(f"tap_{name}", list(t.shape),
                                    t.dtype, kind="ExternalOutput")
                nc.sync.dma_start(td[:], t[:])

    nc.compile()
    return nc


def _host_prep(inputs):
    """Host-side preprocessing shared by all cores + per-core arrays."""
    x = np.asarray(inputs["x"], np.float32)
    mask = np.asarray(inputs["mask"])
    nbr_idx = np.asarray(inputs["nbr_idx"]).astype(np.int64)
    nbr_mask = np.asarray(inputs["nbr_mask"])
    rel_pos = np.asarray(inputs["rel_pos"]).astype(np.int64)

    if not (np.all(mask == 1)):
        raise NotImplementedError("kernel assumes mask == ones (spec fill)")

    # edge-bias table over the 65 possible rel values
    Erel = np.asarray(inputs["Erel"], np.float32)
    We1 = np.asarray(inputs["We1"], np.float32)
    be1 = np.asarray(inputs["be1"], np.float32)
    We2 = np.asarray(inputs["We2"], np.float32)
    be2 = np.asarray(inputs["be2"], np.float32)
    tab = (_gelu_np(Erel @ We1 + be1) @ We2 + be2)[:, 0]  # [65]

    rel = np.clip(rel_pos, -CLIP, CLIP) + CLIP
    ev = np.exp(tab[rel]) * (nbr_mask != 0)  # [B, L, K]

    # dense E^T per batch: ET[b][j, t] = sum_k ev[b,t,k] * [idx==j]
    ET = np.zeros((B, L, L), np.float32)
    for b in range(B):
        t_idx = np.repeat(np.arange(L), K)
        np.add.at(ET[b], (nbr_idx[b].ravel(), t_idx), ev[b].ravel())

    aff = np.zeros((128, 2, 4), np.float32)
    for name, i in (("g1", 0), ("b1", 1), ("g2", 2), ("b2", 3)):
        v = np.asarray(inputs[name], np.float32)
        aff[:, :, i] = v.reshape(2, 128).T

    shared = {
        "aff": aff,
        "wq": _w_tiles(np.asarray(inputs["Wq"], np.float32) / 16.0, 2),
        "wk": _w_tiles(np.asarray(inputs["Wk"], np.float32), 2),
        "wv": _w_tiles(np.asarray(inputs["Wv"], np.float32)
                       @ np.asarray(inputs["Wlo"], np.float32), 2),
        "wg1": _w_tiles(np.asarray(inputs["Wg1"], np.float32), 4),
        "wg2": _w_tiles(np.asarray(inputs["Wg2"], np.float32), 2),
        "wqkv": _w_tiles(np.asarray(inputs["Wqkv"], np.float32), 2),
        "wgo": _w_tiles(np.asarray(inputs["Wgo"], np.float32), 2),
        "wf1": _w_tiles(np.asarray(inputs["Wf1"], np.float32), 2),
        "wf2": _w_tiles(np.asarray(inputs["Wf2"], np.float32), 2),
        "wff1": _w_tiles(np.asarray(inputs["g2"], np.float32)[:, None]
                         * np.asarray(inputs["Wff1"], np.float32), 2),
        "wff2": _w_tiles(np.asarray(inputs["Wff2"], np.float32), 8),
        "pm": np.array([[1.0], [-1.0]], BF16),
    }
    for k in ("blo", "bg1", "bg2", "bf1", "bf2", "bff1", "bff2", "b2"):
        if not np.allclose(np.asarray(inputs[k]), 0.0):
            raise NotImplementedError(f"kernel assumes bias {k} == 0 (spec fill)")

    per_core = []
    for c in range(NCORES):
        b, s = c // SPB, c % SPB
        s0 = s * SH
        xp = np.roll(x[b], -s0, axis=0)
        etp = np.roll(ET[b][:, s0:s0 + SH], -s0, axis=0).astype(BF16)
        per_core.append({"x": np.ascontiguousarray(xp),
                         "et": np.ascontiguousarray(etp)})
    return shared, per_core


def kernel(**inputs) -> np.ndarray:
    import concourse.bass_utils as bu

    if "nc" not in _CACHE:
        _CACHE["nc"] = _build()
    nc = _CACHE["nc"]

    shared, per_core = _host_prep(inputs)
    in_maps = [{**shared, **pc} for pc in per_core]
    res = bu.run_bass_kernel_spmd(nc, in_maps, core_ids=list(range(NCORES)))
    out = np.zeros((B, L, D), np.float32)
    for c in range(NCORES):
        b, s = c // SPB, c % SPB
        out[b, s * SH:(s + 1) * SH] = res.results[c]["out"]
    return out
